# revision 13
# baseline (speedup 1.0000x reference)
"""Bottom-k cross-entropy loss on 8 Trainium2 NeuronCores (v4).

Per-sample CE over [8192, 32000] logits, then mean of the 4096 smallest
losses.  Data-parallel: rows sharded across 8 cores; each core streams its
131MB shard once (memory-bound) through one fused exp+accumulate pass on
the scalar engine, alternating chunk loads between the two HWDGE rings.

Selection runs in y-space (y = sumexp * exp(-picked) = exp(ce)) against
host-exponentiated dyadic thresholds, so the stream never needs a mid-pass
Ln (no ACT table switches).  Blocks 0-5 are all-gathered at ~75% of the
stream (fully hidden; a dummy start collective absorbs launch skew).  The
last two blocks skip the collective entirely: each core replicates its
256 values across partitions and pushes them straight into the other 7
cores' SBUF with remote_dma_broadcast (XOR slot k <- core me^k), so the
final exchange costs ~5us instead of a ~25us ncfw all-gather.

Tail compute is split across DVE and ACT: round-2 counting = DVE is_le on
low columns + ACT sign on high columns; final bottom-m sum = DVE min-accum
on the first half + ACT relu-accum on the second half
(res*m = sum_A min(v,t) - sum_B relu(t-v) with |A| = m).
"""

import numpy as np

N_CORES = 8
N_FULL, V_FULL = 8192, 32000
P = 128

# bracket steps: round-1 range 32, round-2 range 0.25; CE values lie in (0, 32]
S1, S2 = 2.0**-2, 2.0**-9
RB_A = 6  # row blocks in the early all-gather; the rest go via remote DMA


def build_nc(n_cores, r, v, f):
    """Build the SPMD Bass program (identical on every core)."""
    from concourse import bass, bacc, mybir, tile

    assert r % P == 0 and v % f == 0 and f % 2 == 0
    rb_n = r // P
    nch = v // f
    ng = r * n_cores
    m = ng // 2
    rb_b = rb_n - RB_A
    na = RB_A * P * n_cores   # values in the early gather (6144)
    wb = rb_b * P             # per-core values in the remote exchange (256)
    f32 = mybir.dt.float32
    add_dep = tile.add_dep_helper

    nc = bacc.Bacc()
    x = nc.declare_dram_parameter("x", [r, v], f32, isOutput=False)
    offs = nc.declare_dram_parameter("offs", [P, rb_n], mybir.dt.int32, isOutput=False)
    e1 = nc.declare_dram_parameter("e1", [P, 1], f32, isOutput=False)
    io2 = nc.declare_dram_parameter("io2", [P, 1], f32, isOutput=False)
    thr = nc.declare_dram_parameter("thr", [1, 1], mybir.dt.int32, isOutput=False)
    out = nc.declare_dram_parameter("out", [1, 1], f32, isOutput=True)

    rsem = nc.alloc_semaphore("rgather")
    lsem = nc.alloc_semaphore("rgather_l")

    with tile.TileContext(nc) as tc:
        with (
            tc.tile_pool(name="dram", bufs=1, space="DRAM") as dpool,
            tc.tile_pool(name="consts", bufs=1) as cpool,
            tc.tile_pool(name="xs", bufs=4) as xpool,
            tc.tile_pool(name="es", bufs=1) as epool,
            tc.tile_pool(name="part", bufs=3) as partpool,
            tc.tile_pool(name="yrow", bufs=1) as yrowpool,
            tc.tile_pool(name="rep", bufs=1) as reppool,
            tc.tile_pool(name="sel", bufs=1) as selpool,
            tc.tile_pool(name="psum", bufs=2, space="PSUM") as ppool,
        ):
            ya_local = dpool.tile([RB_A * P, 1], f32, name="ya_local")
            yb_local = dpool.tile([wb, 1], f32, name="yb_local")
            ya_all = dpool.tile([na, 1], f32, addr_space="Shared", name="ya_all")
            d_local = dpool.tile([8, 1], f32, name="d_local")
            d_all = dpool.tile([8 * n_cores, 1], f32, addr_space="Shared", name="d_all")

            offs_sb = cpool.tile([P, rb_n], mybir.dt.int32)
            nc.gpsimd.dma_start(offs_sb[:], offs[:])
            e1_sb = cpool.tile([P, 1], f32)
            nc.gpsimd.dma_start(e1_sb[:], e1[:])
            io2_sb = cpool.tile([P, 1], f32)
            nc.gpsimd.dma_start(io2_sb[:], io2[:])
            thr_sb = cpool.tile([1, 1], mybir.dt.int32)
            nc.gpsimd.dma_start(thr_sb[:], thr[:])

            # dummy all-gather: syncs the cores right after launch (absorbing
            # launch skew off the critical path).  Output unread.
            d_sb = cpool.tile([1, 8], f32)
            nc.vector.memset(d_sb[:], 0.0)
            nc.gpsimd.dma_start(d_local[:].rearrange("a 1 -> 1 a"), d_sb[:])
            nc.gpsimd.collective_compute(
                "AllGather",
                mybir.AluOpType.bypass,
                replica_groups=[list(range(n_cores))],
                ins=[d_local[:].opt()],
                outs=[d_all[:].opt()],
            )

            # tiny dummy partition_broadcast: forces the gpsimd ucode library
            # load to happen here (gpsimd is idle during streaming) instead of
            # in the latency-critical tail
            dsrc = cpool.tile([1, 4], f32)
            nc.vector.memset(dsrc[:], 0.0)
            dout = cpool.tile([P, 4], f32)
            nc.gpsimd.partition_broadcast(dout[:], dsrc[:])

            # gather picked logits: x.flat[row*v + label] for each local row
            picked = cpool.tile([P, rb_n], f32)
            x_flat = x[:].rearrange("a b -> (a b) ()")
            for rbi in range(rb_n):
                nc.gpsimd.indirect_dma_start(
                    out=picked[:, rbi : rbi + 1],
                    out_offset=None,
                    in_=x_flat,
                    in_offset=bass.IndirectOffsetOnAxis(
                        ap=offs_sb[:, rbi : rbi + 1], axis=0
                    ),
                )
            # exp(-picked), used to fold the picked logit into y per block
            expnp = cpool.tile([P, rb_n], f32)
            nc.scalar.activation(
                out=expnp[:], in_=picked[:],
                func=mybir.ActivationFunctionType.Exp, scale=-1.0,
            )

            ys = cpool.tile([P, rb_n], f32)
            # replicated values: cols [0:na]=blocks 0-5 (rank-major from the
            # all-gather); cols [na + wb*k] = blocks 6-7 of core (me XOR k)
            xrep = reppool.tile([P, ng], f32, name="xrep")
            lnrep = reppool.tile([P, ng], f32, name="lnrep")
            dummy = selpool.tile([P, 1], f32)
            ones = selpool.tile([P, P], f32)
            nc.vector.memset(ones[:], 1.0)
            ca_a = selpool.tile([P, 1], f32)
            ca_a2 = selpool.tile([P, 1], f32)

            def count_le(dst, cols_lo, cols_hi, thr_ap):
                n_cols = cols_hi - cols_lo
                return nc.vector.tensor_scalar(
                    out=dummy[:].broadcast_to([P, n_cols]),
                    in0=xrep[:, cols_lo:cols_hi],
                    scalar1=thr_ap,
                    scalar2=None,
                    op0=mybir.AluOpType.is_le,
                    op1=mybir.AluOpType.add,
                    accum_out=dst[:],
                )

            # streaming pass: pure DMA + fused exp/accumulate.  Chunk loads
            # alternate between the two HWDGE rings.  The last block streams
            # its final two chunks at half width so the trailing exp (which
            # gates block 7's y) finishes ~1.8us sooner.
            qi = 0
            last_exp = None
            for rbi in range(rb_n):
                if rbi < rb_n - 1:
                    sizes = [f] * nch
                else:
                    sizes = [f] * (nch - 1) + [f // 2, f // 2]
                part = partpool.tile([P, len(sizes)], f32, tag="part", name=f"part{rbi}")
                off = 0
                for ci, sz in enumerate(sizes):
                    xt = xpool.tile([P, sz], f32, tag="xt")
                    eng = nc.sync if qi % 2 == 0 else nc.scalar
                    qi += 1
                    eng.dma_start(
                        xt[:], x[rbi * P : (rbi + 1) * P, off : off + sz]
                    )
                    off += sz
                    esc = epool.tile([P, sz], f32, tag="esc")
                    exp_i = nc.scalar.activation(
                        out=esc[:],
                        in_=xt[:],
                        func=mybir.ActivationFunctionType.Exp,
                        accum_out=part[:, ci : ci + 1],
                    )
                    if rbi == rb_n - 1 and ci == len(sizes) - 1:
                        last_exp = exp_i.ins
                # per-block epilogue (DVE only): y_b = sum(part) * exp(-picked)
                s_b = selpool.tile([P, 1], f32, name=f"s{rbi}", tag="sblk")
                nc.vector.tensor_reduce(
                    s_b[:], part[:], axis=mybir.AxisListType.X,
                    op=mybir.AluOpType.add,
                )
                nc.vector.tensor_tensor(
                    out=ys[:, rbi : rbi + 1], in0=s_b[:],
                    in1=expnp[:, rbi : rbi + 1], op=mybir.AluOpType.mult,
                )

                if rbi == RB_A - 1:
                    # early gather of blocks 0..5 (hidden under the stream
                    # tail).  All DMAs on SWDGE/gpsimd so the stream rings are
                    # never head-of-line blocked.
                    nc.gpsimd.dma_start(
                        ya_local[:].rearrange("(p b) 1 -> p b", b=RB_A),
                        ys[:, :RB_A],
                    )
                    nc.gpsimd.collective_compute(
                        "AllGather",
                        mybir.AluOpType.bypass,
                        replica_groups=[list(range(n_cores))],
                        ins=[ya_local[:].opt()],
                        outs=[ya_all[:].opt()],
                    )
                    ya_row = yrowpool.tile([1, na], f32, tag="yrow", name="ya_row")
                    nc.gpsimd.dma_start(ya_row[:], ya_all[:].rearrange("a 1 -> 1 a"))
                    nc.gpsimd.partition_broadcast(
                        xrep[:, : na // 2], ya_row[:, : na // 2]
                    )
                    nc.gpsimd.partition_broadcast(
                        xrep[:, na // 2 : na], ya_row[:, na // 2 :]
                    )
                    # round-1 counts over the gathered 6/8 (idle DVE)
                    count_le(ca_a, 0, na // 2, e1_sb[:])
                    count_le(ca_a2, na // 2, na, e1_sb[:])
                    nc.vector.tensor_tensor(
                        out=ca_a[:], in0=ca_a[:], in1=ca_a2[:],
                        op=mybir.AluOpType.add,
                    )

            # ---- end of streaming: remote-exchange blocks 6-7 and select ----
            # stage own 256 values, replicate across partitions, then push the
            # replicated tile into all 7 peers' SBUF (slot k <- core me^k)
            nc.gpsimd.dma_start(
                yb_local[:].rearrange("(p b) 1 -> p b", b=rb_b), ys[:, RB_A:]
            )
            yb_row = yrowpool.tile([1, wb], f32, tag="yrow", name="yb_row")
            nc.gpsimd.dma_start(yb_row[:], yb_local[:].rearrange("a 1 -> 1 a"))
            nc.gpsimd.partition_broadcast(xrep[:, na : na + wb], yb_row[:])
            for k in range(1, n_cores):
                rdests = [None] * 8
                rdests[k] = (0, k)
                nc.gpsimd.remote_dma_broadcast(
                    out_ap=xrep[:, na + k * wb : na + (k + 1) * wb],
                    in_ap=xrep[:, na : na + wb],
                    remote_sem=rsem,
                    local_sem=lsem,
                    rdests=rdests,
                )
            trig = nc.gpsimd.trigger_dma(count=None)
            # wait for the 7 incoming transfers (2 rsem incs each).  The
            # threshold is loaded from an input tensor: the single-core
            # scheduling sim can't see remote increments and would call a
            # literal >=14 a deadlock.
            thr_reg = nc.gpsimd.alloc_register("thr_reg")
            ld_i = nc.gpsimd.reg_load(thr_reg, thr_sb[:])
            wait_i = nc.gpsimd.wait_ge(rsem, thr_reg)
            add_dep(wait_i.ins, ld_i.ins, sync=False, reason="wait after thr load")
            add_dep(wait_i.ins, trig.ins, sync=False, reason="wait after trigger")

            # ln of the gathered 6/8: pinned on ACT right after the last
            # stream exp so it hides under the remote exchange
            ln_a_i = nc.scalar.activation(
                out=lnrep[:, :na], in_=xrep[:, :na],
                func=mybir.ActivationFunctionType.Ln,
            )
            add_dep(ln_a_i.ins, last_exp, sync=False, reason="ln_a after stream")
            ln_b_i = nc.scalar.activation(
                out=lnrep[:, na:], in_=xrep[:, na:],
                func=mybir.ActivationFunctionType.Ln,
            )
            add_dep(ln_b_i.ins, ln_a_i.ins, sync=False, reason="ln_b after ln_a")
            add_dep(ln_b_i.ins, wait_i.ins, sync=True, reason="ln_b after remote")

            # round 1 finish: counts over the remote-exchanged 2/8
            ca_b = selpool.tile([P, 1], f32)
            cab_i = count_le(ca_b, na, ng, e1_sb[:])
            add_dep(cab_i.ins, wait_i.ins, sync=True, reason="count after remote")
            c1 = selpool.tile([P, 1], f32)
            nc.vector.tensor_tensor(
                out=c1[:], in0=ca_a[:], in1=ca_b[:], op=mybir.AluOpType.add
            )
            ge1 = selpool.tile([P, 1], f32)
            nc.vector.tensor_scalar(
                out=ge1[:], in0=c1[:], scalar1=float(m), scalar2=None,
                op0=mybir.AluOpType.is_ge,
            )
            g1 = ppool.tile([P, 1], f32, name="g1", tag="gps")
            nc.tensor.matmul(out=g1[:], lhsT=ones[:], rhs=ge1[:], start=True, stop=True)
            lo1 = selpool.tile([P, 1], f32)
            nc.vector.tensor_scalar(
                out=lo1[:], in0=g1[:], scalar1=-S1, scalar2=None,
                op0=mybir.AluOpType.mult,
            )
            # round-2 thresholds in y-space: E2[p] = exp(lo1 + 32 + (p+1)*S2)
            arg2 = selpool.tile([P, 1], f32)
            nc.vector.tensor_tensor(
                out=arg2[:], in0=lo1[:], in1=io2_sb[:], op=mybir.AluOpType.add
            )
            e2 = selpool.tile([P, 1], f32)
            e2_i = nc.scalar.activation(
                out=e2[:], in_=arg2[:], func=mybir.ActivationFunctionType.Exp
            )
            add_dep(e2_i.ins, ln_b_i.ins, sync=False, reason="e2 after ln_b")

            # round 2 counts over all values, split DVE (is_le, low cols) +
            # ACT (sign, high cols): sg = sum sign(E2 - y) = #le - #gt
            ndve = 4608
            nact = ng - ndve
            c2d = selpool.tile([P, 1], f32)
            count_le(c2d, 0, ndve, e2[:])
            scr = epool.tile([P, nact], f32, tag="esc", name="scr_sg")
            sg2 = selpool.tile([P, 1], f32)
            sg_i = nc.scalar.activation(
                out=scr[:],
                in_=xrep[:, ndve:],
                func=mybir.ActivationFunctionType.Sign,
                bias=e2[:],
                scale=-1.0,
                accum_out=sg2[:],
            )
            add_dep(sg_i.ins, e2_i.ins, sync=False, reason="sign after e2")
            add_dep(sg_i.ins, wait_i.ins, sync=True, reason="sign after remote")
            # count >= m  <=>  c2d + (nact + sg2)/2 >= m
            u2 = selpool.tile([P, 1], f32)
            nc.vector.tensor_scalar(
                out=u2[:], in0=sg2[:], scalar1=0.5, scalar2=c2d[:],
                op0=mybir.AluOpType.mult, op1=mybir.AluOpType.add,
            )
            ge2 = selpool.tile([P, 1], f32)
            nc.vector.tensor_scalar(
                out=ge2[:], in0=u2[:], scalar1=float(m) - nact / 2.0, scalar2=None,
                op0=mybir.AluOpType.is_ge,
            )
            g2 = ppool.tile([P, 1], f32, name="g2", tag="gps")
            nc.tensor.matmul(out=g2[:], lhsT=ones[:], rhs=ge2[:], start=True, stop=True)
            lo2 = selpool.tile([P, 1], f32)
            nc.vector.tensor_scalar(
                out=lo2[:], in0=g2[:], scalar1=-S2, scalar2=lo1[:],
                op0=mybir.AluOpType.mult, op1=mybir.AluOpType.add,
            )
            # final threshold t = lo2 + (range1 + range2 + S2); t >= v_(m)
            # within one S2 bracket
            c_t = 128.0 * S1 + 128.0 * S2 + S2
            tf = selpool.tile([P, 1], f32)
            nc.vector.tensor_scalar(
                out=tf[:], in0=lo2[:], scalar1=c_t, scalar2=None,
                op0=mybir.AluOpType.add,
            )
            # bottom-m mean, split DVE/ACT with |A| = m:
            #   res*m = sum_A min(v,t) - sum_B relu(t-v)
            sm_a = selpool.tile([P, 1], f32)
            nc.vector.tensor_scalar(
                out=dummy[:].broadcast_to([P, m]),
                in0=lnrep[:, :m],
                scalar1=tf[:],
                scalar2=None,
                op0=mybir.AluOpType.min,
                op1=mybir.AluOpType.add,
                accum_out=sm_a[:],
            )
            scr2 = epool.tile([P, ng - m], f32, tag="esc", name="scr_relu")
            sr_b = selpool.tile([P, 1], f32)
            relu_i = nc.scalar.activation(
                out=scr2[:],
                in_=lnrep[:, m:],
                func=mybir.ActivationFunctionType.Relu,
                bias=tf[:],
                scale=-1.0,
                accum_out=sr_b[:],
            )
            add_dep(relu_i.ins, sg_i.ins, sync=False, reason="relu after sign")
            d = selpool.tile([P, 1], f32)
            nc.vector.tensor_tensor(
                out=d[:], in0=sm_a[:], in1=sr_b[:], op=mybir.AluOpType.subtract
            )
            res = selpool.tile([P, 1], f32)
            nc.vector.tensor_scalar(
                out=res[:], in0=d[:], scalar1=1.0 / m, scalar2=None,
                op0=mybir.AluOpType.mult,
            )
            nc.sync.dma_start(out[:], res[0:1, :])

    if not nc.is_finalized():
        nc.finalize()
    return nc


def make_host_inputs(x_full, labels_full, n_cores, r, v):
    """Shard rows across cores and build the per-core input maps."""
    rb_n = r // P
    e1 = np.exp((np.arange(P, dtype=np.float64) + 1) * S1).astype(np.float32)
    io2 = (128 * S1 + (np.arange(P, dtype=np.float64) + 1) * S2).astype(np.float32)
    in_maps = []
    for c in range(n_cores):
        rows = slice(c * r, (c + 1) * r)
        xs = np.ascontiguousarray(x_full[rows], dtype=np.float32)
        lb = np.asarray(labels_full[rows], dtype=np.int64)
        offs_flat = (np.arange(r, dtype=np.int64) * v + lb).astype(np.int32)
        offs = np.ascontiguousarray(offs_flat.reshape(rb_n, P).T)
        in_maps.append(
            {
                "x": xs,
                "offs": offs,
                "e1": e1.reshape(P, 1),
                "io2": io2.reshape(P, 1),
                "thr": np.array([[14]], dtype=np.int32),
            }
        )
    return in_maps


def run(inputs, trace=False, f=4000):
    from concourse.bass_utils import run_bass_kernel_spmd

    x_full = np.asarray(inputs["outputs"], dtype=np.float32)
    labels_full = np.asarray(inputs["labels"])
    n, v = x_full.shape
    r = n // N_CORES
    nc = build_nc(N_CORES, r, v, f)
    in_maps = make_host_inputs(x_full, labels_full, N_CORES, r, v)
    try:
        res = run_bass_kernel_spmd(
            nc, in_maps, list(range(N_CORES)), trace=trace
        )
    except Exception:
        # transient device errors (e.g. a wedged core from a prior run)
        # usually clear on retry
        res = run_bass_kernel_spmd(
            nc, in_maps, list(range(N_CORES)), trace=trace
        )
    val = np.asarray(res.results[0]["out"], dtype=np.float32).reshape(-1)[0]
    return np.asarray(val, dtype=np.float32), res


def kernel(outputs=None, labels=None, **_ignored):
    out, _ = run({"outputs": outputs, "labels": labels})
    return out


# revision 14
# speedup vs baseline: 1.0501x; 1.0501x over previous
"""Bottom-k cross-entropy loss on 8 Trainium2 NeuronCores (v5).

Per-sample CE over [8192, 32000] logits, then mean of the 4096 smallest
losses.  Data-parallel: rows sharded across 8 cores; each core streams its
131MB shard once (memory-bound) through one fused exp+accumulate pass on
the scalar engine, alternating 3.28MB chunk loads between the two HWDGE
rings (bigger chunks -> better SDMA efficiency than 2MB).

Selection runs in y-space (y = sumexp * exp(-picked) = exp(ce)) against
host-exponentiated dyadic thresholds, so the stream never needs a mid-pass
Ln (no ACT table switches).  Blocks 0-5 are all-gathered at ~75% of the
stream (fully hidden; a dummy start collective absorbs launch skew); only
blocks 6-7 (2048 values) gather on the critical path at the end.

Tail compute is split across DVE and ACT: round-2 counting = DVE is_le on
low columns + ACT sign on high columns; final bottom-m sum = DVE min-accum
on the first half + ACT relu-accum on the second half
(res*m = sum_A min(v,t) - sum_B relu(t-v) with |A| = m).  The gathered
row staging buffers alias partition-0 rows of lnrep (dead until the
post-stream Ln) to keep SBUF under budget at the bigger chunk size.
"""

import numpy as np

N_CORES = 8
N_FULL, V_FULL = 8192, 32000
P = 128

# bracket steps: round-1 range 32, round-2 range 0.25; CE values lie in (0, 32]
S1, S2 = 2.0**-2, 2.0**-9
RB_A = 6  # row blocks in the early all-gather


def build_nc(n_cores, r, v, f):
    """Build the SPMD Bass program (identical on every core)."""
    from concourse import bass, bacc, mybir, tile

    assert r % P == 0 and v % f == 0 and f % 2 == 0
    rb_n = r // P
    nch = v // f
    ng = r * n_cores
    m = ng // 2
    rb_b = rb_n - RB_A
    na = RB_A * P * n_cores   # values in the early gather (6144)
    nb = rb_b * P * n_cores   # values in the final gather (2048)
    f32 = mybir.dt.float32
    add_dep = tile.add_dep_helper

    nc = bacc.Bacc()
    x = nc.declare_dram_parameter("x", [r, v], f32, isOutput=False)
    offs = nc.declare_dram_parameter("offs", [P, rb_n], mybir.dt.int32, isOutput=False)
    e1 = nc.declare_dram_parameter("e1", [P, 1], f32, isOutput=False)
    io2 = nc.declare_dram_parameter("io2", [P, 1], f32, isOutput=False)
    out = nc.declare_dram_parameter("out", [1, 1], f32, isOutput=True)

    with tile.TileContext(nc) as tc:
        with (
            tc.tile_pool(name="dram", bufs=1, space="DRAM") as dpool,
            tc.tile_pool(name="consts", bufs=1) as cpool,
            tc.tile_pool(name="xs", bufs=3) as xpool,
            tc.tile_pool(name="es", bufs=1) as epool,
            tc.tile_pool(name="part", bufs=3) as partpool,
            tc.tile_pool(name="rep", bufs=1) as reppool,
            tc.tile_pool(name="sel", bufs=1) as selpool,
            tc.tile_pool(name="psum", bufs=2, space="PSUM") as ppool,
        ):
            ya_local = dpool.tile([RB_A * P, 1], f32, name="ya_local")
            yb_local = dpool.tile([rb_b * P, 1], f32, name="yb_local")
            ya_all = dpool.tile([na, 1], f32, addr_space="Shared", name="ya_all")
            yb_all = dpool.tile([nb, 1], f32, addr_space="Shared", name="yb_all")
            d_local = dpool.tile([8, 1], f32, name="d_local")
            d_all = dpool.tile([8 * n_cores, 1], f32, addr_space="Shared", name="d_all")

            offs_sb = cpool.tile([P, rb_n], mybir.dt.int32)
            nc.gpsimd.dma_start(offs_sb[:], offs[:])
            e1_sb = cpool.tile([P, 1], f32)
            nc.gpsimd.dma_start(e1_sb[:], e1[:])
            io2_sb = cpool.tile([P, 1], f32)
            nc.gpsimd.dma_start(io2_sb[:], io2[:])

            # dummy all-gather: syncs the cores right after launch (absorbing
            # launch skew off the critical path).  Output unread.
            d_sb = cpool.tile([1, 8], f32)
            nc.vector.memset(d_sb[:], 0.0)
            nc.gpsimd.dma_start(d_local[:].rearrange("a 1 -> 1 a"), d_sb[:])
            nc.gpsimd.collective_compute(
                "AllGather",
                mybir.AluOpType.bypass,
                replica_groups=[list(range(n_cores))],
                ins=[d_local[:].opt()],
                outs=[d_all[:].opt()],
            )

            # tiny dummy partition_broadcast: forces the gpsimd ucode library
            # load to happen here (gpsimd is idle during streaming) instead of
            # in the latency-critical tail
            dsrc = cpool.tile([1, 4], f32)
            nc.vector.memset(dsrc[:], 0.0)
            dout = cpool.tile([P, 4], f32)
            nc.gpsimd.partition_broadcast(dout[:], dsrc[:])

            # gather picked logits: x.flat[row*v + label] for each local row
            picked = cpool.tile([P, rb_n], f32)
            x_flat = x[:].rearrange("a b -> (a b) ()")
            for rbi in range(rb_n):
                nc.gpsimd.indirect_dma_start(
                    out=picked[:, rbi : rbi + 1],
                    out_offset=None,
                    in_=x_flat,
                    in_offset=bass.IndirectOffsetOnAxis(
                        ap=offs_sb[:, rbi : rbi + 1], axis=0
                    ),
                )
            # exp(-picked), used to fold the picked logit into y per block
            expnp = cpool.tile([P, rb_n], f32)
            nc.scalar.activation(
                out=expnp[:], in_=picked[:],
                func=mybir.ActivationFunctionType.Exp, scale=-1.0,
            )

            ys = cpool.tile([P, rb_n], f32)
            # replicated values: cols [0:na]=blocks 0-5 (rank-major),
            # [na:ng]=blocks 6-7 (rank-major)
            xrep = reppool.tile([P, ng], f32, name="xrep")
            lnrep = reppool.tile([P, ng], f32, name="lnrep")
            dummy = selpool.tile([P, 1], f32)
            ones = selpool.tile([P, P], f32)
            nc.vector.memset(ones[:], 1.0)
            ca_a = selpool.tile([P, 1], f32)
            ca_a2 = selpool.tile([P, 1], f32)

            def count_le(dst, cols_lo, cols_hi, thr_ap):
                n_cols = cols_hi - cols_lo
                return nc.vector.tensor_scalar(
                    out=dummy[:].broadcast_to([P, n_cols]),
                    in0=xrep[:, cols_lo:cols_hi],
                    scalar1=thr_ap,
                    scalar2=None,
                    op0=mybir.AluOpType.is_le,
                    op1=mybir.AluOpType.add,
                    accum_out=dst[:],
                )

            # streaming pass: pure DMA + fused exp/accumulate.  Chunk loads
            # alternate between the two HWDGE rings.  The last block streams
            # its final two chunks at half width so the trailing exp (which
            # gates block 7's y) finishes sooner.
            qi = 0
            last_exp = None
            for rbi in range(rb_n):
                if rbi < rb_n - 1:
                    sizes = [f] * nch
                else:
                    sizes = [f] * (nch - 1) + [f // 2, f // 2]
                part = partpool.tile([P, len(sizes)], f32, tag="part", name=f"part{rbi}")
                off = 0
                for ci, sz in enumerate(sizes):
                    xt = xpool.tile([P, sz], f32, tag="xt")
                    eng = nc.sync if qi % 2 == 0 else nc.scalar
                    qi += 1
                    eng.dma_start(
                        xt[:], x[rbi * P : (rbi + 1) * P, off : off + sz]
                    )
                    off += sz
                    esc = epool.tile([P, sz], f32, tag="esc")
                    exp_i = nc.scalar.activation(
                        out=esc[:],
                        in_=xt[:],
                        func=mybir.ActivationFunctionType.Exp,
                        accum_out=part[:, ci : ci + 1],
                    )
                    if rbi == rb_n - 1 and ci == len(sizes) - 1:
                        last_exp = exp_i.ins
                # per-block epilogue (DVE only): y_b = sum(part) * exp(-picked)
                s_b = selpool.tile([P, 1], f32, name=f"s{rbi}", tag="sblk")
                nc.vector.tensor_reduce(
                    s_b[:], part[:], axis=mybir.AxisListType.X,
                    op=mybir.AluOpType.add,
                )
                nc.vector.tensor_tensor(
                    out=ys[:, rbi : rbi + 1], in0=s_b[:],
                    in1=expnp[:, rbi : rbi + 1], op=mybir.AluOpType.mult,
                )

                if rbi == RB_A - 1:
                    # early gather of blocks 0..5 (hidden under the stream
                    # tail).  All DMAs on SWDGE/gpsimd so the stream rings are
                    # never head-of-line blocked.  The [1, na] row stages into
                    # partition 0 of lnrep, which is dead until the
                    # post-stream Ln overwrites it.
                    nc.gpsimd.dma_start(
                        ya_local[:].rearrange("(p b) 1 -> p b", b=RB_A),
                        ys[:, :RB_A],
                    )
                    nc.gpsimd.collective_compute(
                        "AllGather",
                        mybir.AluOpType.bypass,
                        replica_groups=[list(range(n_cores))],
                        ins=[ya_local[:].opt()],
                        outs=[ya_all[:].opt()],
                    )
                    ya_row = lnrep[0:1, :na]
                    nc.gpsimd.dma_start(ya_row, ya_all[:].rearrange("a 1 -> 1 a"))
                    nc.gpsimd.partition_broadcast(
                        xrep[:, : na // 2], lnrep[0:1, : na // 2]
                    )
                    nc.gpsimd.partition_broadcast(
                        xrep[:, na // 2 : na], lnrep[0:1, na // 2 : na]
                    )
                    # round-1 counts over the gathered 6/8 (idle DVE)
                    count_le(ca_a, 0, na // 2, e1_sb[:])
                    count_le(ca_a2, na // 2, na, e1_sb[:])
                    nc.vector.tensor_tensor(
                        out=ca_a[:], in0=ca_a[:], in1=ca_a2[:],
                        op=mybir.AluOpType.add,
                    )

            # ---- end of streaming: gather blocks 6-7 and select ----
            nc.gpsimd.dma_start(
                yb_local[:].rearrange("(p b) 1 -> p b", b=rb_b), ys[:, RB_A:]
            )
            nc.gpsimd.collective_compute(
                "AllGather",
                mybir.AluOpType.bypass,
                replica_groups=[list(range(n_cores))],
                ins=[yb_local[:].opt()],
                outs=[yb_all[:].opt()],
            )

            # ln of the gathered 6/8: pinned on ACT right after the last
            # stream exp so it hides under the final all-gather
            ln_a_i = nc.scalar.activation(
                out=lnrep[:, :na], in_=xrep[:, :na],
                func=mybir.ActivationFunctionType.Ln,
            )
            add_dep(ln_a_i.ins, last_exp, sync=False, reason="ln_a after stream")

            yb_row = lnrep[0:1, na:]
            nc.sync.dma_start(yb_row, yb_all[:].rearrange("a 1 -> 1 a"))
            nc.gpsimd.partition_broadcast(xrep[:, na:], lnrep[0:1, na:])

            # round 1 finish: counts over blocks 6-7
            ca_b = selpool.tile([P, 1], f32)
            count_le(ca_b, na, ng, e1_sb[:])
            c1 = selpool.tile([P, 1], f32)
            nc.vector.tensor_tensor(
                out=c1[:], in0=ca_a[:], in1=ca_b[:], op=mybir.AluOpType.add
            )
            ge1 = selpool.tile([P, 1], f32)
            nc.vector.tensor_scalar(
                out=ge1[:], in0=c1[:], scalar1=float(m), scalar2=None,
                op0=mybir.AluOpType.is_ge,
            )
            g1 = ppool.tile([P, 1], f32, name="g1", tag="gps")
            nc.tensor.matmul(out=g1[:], lhsT=ones[:], rhs=ge1[:], start=True, stop=True)
            lo1 = selpool.tile([P, 1], f32)
            nc.vector.tensor_scalar(
                out=lo1[:], in0=g1[:], scalar1=-S1, scalar2=None,
                op0=mybir.AluOpType.mult,
            )
            # round-2 thresholds in y-space: E2[p] = exp(lo1 + 32 + (p+1)*S2)
            arg2 = selpool.tile([P, 1], f32)
            nc.vector.tensor_tensor(
                out=arg2[:], in0=lo1[:], in1=io2_sb[:], op=mybir.AluOpType.add
            )
            e2 = selpool.tile([P, 1], f32)
            e2_i = nc.scalar.activation(
                out=e2[:], in_=arg2[:], func=mybir.ActivationFunctionType.Exp
            )
            add_dep(e2_i.ins, ln_a_i.ins, sync=False, reason="e2 after ln_a")

            # round 2 counts over all values, split DVE (is_le, low cols) +
            # ACT (sign, high cols): sign(E2 - y) summed = #le - #gt
            ndve = 4608
            nact = ng - ndve
            c2d = selpool.tile([P, 1], f32)
            count_le(c2d, 0, ndve, e2[:])
            scr = epool.tile([P, nact], f32, tag="esc", name="scr_sg")
            sg2 = selpool.tile([P, 1], f32)
            sg_i = nc.scalar.activation(
                out=scr[:],
                in_=xrep[:, ndve:],
                func=mybir.ActivationFunctionType.Sign,
                bias=e2[:],
                scale=-1.0,
                accum_out=sg2[:],
            )
            add_dep(sg_i.ins, e2_i.ins, sync=False, reason="sign after e2")
            ln_b_i = nc.scalar.activation(
                out=lnrep[:, na:], in_=xrep[:, na:],
                func=mybir.ActivationFunctionType.Ln,
            )
            add_dep(ln_b_i.ins, sg_i.ins, sync=False, reason="ln_b after sign")
            # count >= m  <=>  c2d + (nact + sg2)/2 >= m
            u2 = selpool.tile([P, 1], f32)
            nc.vector.tensor_scalar(
                out=u2[:], in0=sg2[:], scalar1=0.5, scalar2=c2d[:],
                op0=mybir.AluOpType.mult, op1=mybir.AluOpType.add,
            )
            ge2 = selpool.tile([P, 1], f32)
            nc.vector.tensor_scalar(
                out=ge2[:], in0=u2[:], scalar1=float(m) - nact / 2.0, scalar2=None,
                op0=mybir.AluOpType.is_ge,
            )
            g2 = ppool.tile([P, 1], f32, name="g2", tag="gps")
            nc.tensor.matmul(out=g2[:], lhsT=ones[:], rhs=ge2[:], start=True, stop=True)
            lo2 = selpool.tile([P, 1], f32)
            nc.vector.tensor_scalar(
                out=lo2[:], in0=g2[:], scalar1=-S2, scalar2=lo1[:],
                op0=mybir.AluOpType.mult, op1=mybir.AluOpType.add,
            )
            # final threshold t = lo2 + (range1 + range2 + S2); t >= v_(m)
            # within one S2 bracket
            c_t = 128.0 * S1 + 128.0 * S2 + S2
            tf = selpool.tile([P, 1], f32)
            nc.vector.tensor_scalar(
                out=tf[:], in0=lo2[:], scalar1=c_t, scalar2=None,
                op0=mybir.AluOpType.add,
            )
            # bottom-m mean, split DVE/ACT with |A| = m:
            #   res*m = sum_A min(v,t) - sum_B relu(t-v)
            sm_a = selpool.tile([P, 1], f32)
            nc.vector.tensor_scalar(
                out=dummy[:].broadcast_to([P, m]),
                in0=lnrep[:, :m],
                scalar1=tf[:],
                scalar2=None,
                op0=mybir.AluOpType.min,
                op1=mybir.AluOpType.add,
                accum_out=sm_a[:],
            )
            scr2 = epool.tile([P, ng - m], f32, tag="esc", name="scr_relu")
            sr_b = selpool.tile([P, 1], f32)
            relu_i = nc.scalar.activation(
                out=scr2[:],
                in_=lnrep[:, m:],
                func=mybir.ActivationFunctionType.Relu,
                bias=tf[:],
                scale=-1.0,
                accum_out=sr_b[:],
            )
            add_dep(relu_i.ins, ln_b_i.ins, sync=False, reason="relu after ln_b")
            d = selpool.tile([P, 1], f32)
            nc.vector.tensor_tensor(
                out=d[:], in0=sm_a[:], in1=sr_b[:], op=mybir.AluOpType.subtract
            )
            res = selpool.tile([P, 1], f32)
            nc.vector.tensor_scalar(
                out=res[:], in0=d[:], scalar1=1.0 / m, scalar2=None,
                op0=mybir.AluOpType.mult,
            )
            nc.sync.dma_start(out[:], res[0:1, :])

    if not nc.is_finalized():
        nc.finalize()
    return nc


def make_host_inputs(x_full, labels_full, n_cores, r, v):
    """Shard rows across cores and build the per-core input maps."""
    rb_n = r // P
    e1 = np.exp((np.arange(P, dtype=np.float64) + 1) * S1).astype(np.float32)
    io2 = (128 * S1 + (np.arange(P, dtype=np.float64) + 1) * S2).astype(np.float32)
    in_maps = []
    for c in range(n_cores):
        rows = slice(c * r, (c + 1) * r)
        xs = np.ascontiguousarray(x_full[rows], dtype=np.float32)
        lb = np.asarray(labels_full[rows], dtype=np.int64)
        offs_flat = (np.arange(r, dtype=np.int64) * v + lb).astype(np.int32)
        offs = np.ascontiguousarray(offs_flat.reshape(rb_n, P).T)
        in_maps.append(
            {
                "x": xs,
                "offs": offs,
                "e1": e1.reshape(P, 1),
                "io2": io2.reshape(P, 1),
            }
        )
    return in_maps


def run(inputs, trace=False, f=6400):
    from concourse.bass_utils import run_bass_kernel_spmd

    x_full = np.asarray(inputs["outputs"], dtype=np.float32)
    labels_full = np.asarray(inputs["labels"])
    n, v = x_full.shape
    r = n // N_CORES
    nc = build_nc(N_CORES, r, v, f)
    in_maps = make_host_inputs(x_full, labels_full, N_CORES, r, v)
    try:
        res = run_bass_kernel_spmd(
            nc, in_maps, list(range(N_CORES)), trace=trace
        )
    except Exception:
        # transient device errors (e.g. a wedged core from a prior run)
        # usually clear on retry
        res = run_bass_kernel_spmd(
            nc, in_maps, list(range(N_CORES)), trace=trace
        )
    val = np.asarray(res.results[0]["out"], dtype=np.float32).reshape(-1)[0]
    return np.asarray(val, dtype=np.float32), res


def kernel(outputs=None, labels=None, **_ignored):
    out, _ = run({"outputs": outputs, "labels": labels})
    return out
